# revision 1
# baseline (speedup 1.0000x reference)
# Trainium2 Bass kernel for nn_Decoder_14568529068506 (gnn_message_passing).
#
# Reference computation (per scene s of 32, P=48 peds):
#   rel[i,j]  = obs[j] - obs[i]                  (P,P,2T)   2T=16
#   emb       = rel @ W_se.T                     (P,P,512)
#   emb      *= tile(traj_weight[s])             (P,P,512)
#   x         = concat([emb, h[j]], -1)          (P,P,576)
#   x1        = relu(x @ W1.T + b1)              (P,P,512)
#   x2        = relu(x1 @ W2.T + b2)             (P,P,1024)
#   out[s,i]  = max_j x2[i,j]                    (P,1024)
#
# Kernel restructuring (validated exactly in fp32 numpy):
#  * The tiled traj_weight multiply + spatial embedding + W1 are fused:
#      out1[d,row] = sum_{(ct,g)} Wf[d,(ct,g)] * tw[row,ct] * rel[row,g]
#    with Wf[d, ct*16+g] = sum_{k%2==c} W1[d, t*64+k] * W_se[t*64+k, g].
#    So MLP1 contracts over 256 "rel2" features (+64 h features) instead
#    of 576, and the (P,P,512) embedding is never materialized.
#  * rel2 = tw_rep * rel_rep is built feature-major on 128 partitions:
#      rel_rep = obs_rep.T @ D   (D = +-1 pairwise difference matrix)
#      tw_rep  = R.T @ twT       (R = 0/1 replication matrix)
#    i.e. three cheap matmuls + two vector multiplies per row block.
#  * The h-state part of MLP1 rides as a third K=64 accumulation matmul
#    whose rhs (h broadcast over i) is built once per scene.
#  * relu/bias commute with max-pool, so MLP2 outputs are max-pooled
#    straight out of PSUM; bias+relu are applied post-pool on [128,48].
#  * Matmuls run in bf16 (1 cycle/row; separate LDWEIGHTS path). PSUM
#    accumulation stays fp32; only matmul operands are rounded.
#
# Sharding: scenes are data-parallel across the 8 cores (4 scenes each);
# weights replicated; the (192,1024) per-core outputs are concatenated on
# the host (no collectives needed).

import numpy as np

S, P, T, E, H = 32, 48, 8, 64, 64
D1, D2 = 512, 1024
B = S * P
NCORES = 8
SC = S // NCORES          # scenes per core
NB = 6                    # row blocks per scene
NBLK = P * P // NB        # 384 columns (pairs) per block = 8 i-groups x 48 j
IB = NBLK // P            # i-groups per block (8)


def _host_constants(W_se, W1, W2, b1, b2):
    """Precompute fused weights + structural constant matrices (fp32)."""
    W_se = np.asarray(W_se, np.float32)
    W1 = np.asarray(W1, np.float32)
    W2 = np.asarray(W2, np.float32)
    b1 = np.asarray(b1, np.float32)
    b2 = np.asarray(b2, np.float32)

    W1e, W1h = W1[:, :512], W1[:, 512:]
    Wf = np.zeros((D1, 256), np.float32)
    for c in range(2):
        for t in range(T):
            ct = c * 8 + t
            f = t * 64 + np.arange(c, 64, 2)
            Wf[:, ct * 16:(ct + 1) * 16] = W1e[:, f] @ W_se[f, :]

    Dm = np.zeros((P, P * P), np.float32)
    ii, jj = np.meshgrid(np.arange(P), np.arange(P), indexing="ij")
    rows = (ii * P + jj).ravel()
    np.add.at(Dm, (jj.ravel(), rows), 1.0)
    np.add.at(Dm, (ii.ravel(), rows), -1.0)

    R0 = np.zeros((16, 128), np.float32)
    R1 = np.zeros((16, 128), np.float32)
    for ct in range(8):
        R0[ct, ct * 16:(ct + 1) * 16] = 1.0
        R1[ct + 8, ct * 16:(ct + 1) * 16] = 1.0

    # lhsT tile layouts: [128, kTiles, M] so DMAs are contiguous
    Wf_sb = np.ascontiguousarray(Wf.T.reshape(2, 128, D1).transpose(1, 0, 2))
    W1h_sb = np.ascontiguousarray(W1h.T)                     # (64, 512)
    W2_sb = np.ascontiguousarray(W2.T.reshape(4, 128, D2).transpose(1, 0, 2))
    b1_sb = np.ascontiguousarray(b1.reshape(4, 128).T)       # (128, 4)
    b2_sb = np.ascontiguousarray(b2.reshape(8, 128).T)       # (128, 8)
    ident = np.eye(128, dtype=np.float32)
    return dict(Wf_sb=Wf_sb, W1h_sb=W1h_sb, W2_sb=W2_sb, b1_sb=b1_sb,
                b2_sb=b2_sb, Dm=Dm, R0=R0, R1=R1, ident=ident)


def build_program(n_scenes=SC):
    """Emit the per-core Bass/Tile program. Returns the compiled Bacc.

    Built on bacc.Bacc (not raw bass.Bass): Bacc.compile() runs the
    TRN2 sync legalization (move_matmul_waits_to_ldweights +
    generate_event_semaphores) that splits multi-semaphore waits —
    hardware allows at most one sync-wait per instruction.
    """
    from contextlib import ExitStack
    import concourse.bacc as bacc
    import concourse.tile as tile
    from concourse import mybir
    from concourse.alu_op_type import AluOpType

    f32 = mybir.dt.float32
    bf16 = mybir.dt.bfloat16
    AF = mybir.ActivationFunctionType
    AX = mybir.AxisListType

    nc = bacc.Bacc("TRN2", target_bir_lowering=False, debug=False)

    # ---- DRAM parameters -------------------------------------------------
    d_obs = nc.dram_tensor("obs_rm", [n_scenes * P, 16], bf16, kind="ExternalInput")
    d_tw = nc.dram_tensor("twT", [n_scenes, 16, P * P], bf16, kind="ExternalInput")
    d_h = nc.dram_tensor("h_fm", [n_scenes, 64, P], bf16, kind="ExternalInput")
    d_Dm = nc.dram_tensor("Dm", [P, P * P], bf16, kind="ExternalInput")
    d_R0 = nc.dram_tensor("R0", [16, 128], bf16, kind="ExternalInput")
    d_R1 = nc.dram_tensor("R1", [16, 128], bf16, kind="ExternalInput")
    d_Wf = nc.dram_tensor("Wf_sb", [128, 2, D1], bf16, kind="ExternalInput")
    d_W1h = nc.dram_tensor("W1h_sb", [64, D1], bf16, kind="ExternalInput")
    d_W2 = nc.dram_tensor("W2_sb", [128, 4, D2], bf16, kind="ExternalInput")
    d_b1 = nc.dram_tensor("b1_sb", [128, 4], f32, kind="ExternalInput")
    d_b2 = nc.dram_tensor("b2_sb", [128, 8], f32, kind="ExternalInput")
    d_id = nc.dram_tensor("ident", [128, 128], f32, kind="ExternalInput")
    d_out = nc.dram_tensor("out", [n_scenes * P, D2], f32, kind="ExternalOutput")

    with ExitStack() as ctx:
        tc = ctx.enter_context(tile.TileContext(nc))
        consts = ctx.enter_context(tc.tile_pool(name="consts", bufs=1))
        tw_pool = ctx.enter_context(tc.tile_pool(name="tw", bufs=2))
        scene_pool = ctx.enter_context(tc.tile_pool(name="scene", bufs=2))
        blk_pool = ctx.enter_context(tc.tile_pool(name="blk", bufs=3))
        pp = ctx.enter_context(tc.tile_pool(name="pp", bufs=2, space="PSUM"))
        p1 = ctx.enter_context(tc.tile_pool(name="p1", bufs=2, space="PSUM"))
        p2 = ctx.enter_context(tc.tile_pool(name="p2", bufs=4, space="PSUM"))

        # ---- small resident constants (big weights stream in after the
        # first scene's data so the first matmuls start sooner) ----------
        Dm_sb = consts.tile([P, P * P], bf16)
        nc.sync.dma_start(Dm_sb[:], d_Dm[:])
        Rb_sb = consts.tile([128, 128], bf16)
        nc.sync.dma_start(Rb_sb[64:80, :], d_R0[:])
        nc.sync.dma_start(Rb_sb[96:112, :], d_R1[:])
        b1_sb = consts.tile([128, 4], f32)
        nc.sync.dma_start(b1_sb[:], d_b1[:])
        b2_sb = consts.tile([128, 8], f32)
        nc.sync.dma_start(b2_sb[:], d_b2[:])
        id_sb = consts.tile([128, 128], f32)
        nc.sync.dma_start(id_sb[:], d_id[:])
        zero_sb = consts.tile([128, P], f32)
        nc.vector.memset(zero_sb[:], 0.0)
        Wf_sb = consts.tile([128, 2, D1], bf16)
        W1h_sb = consts.tile([64, D1], bf16)
        W2_sb = consts.tile([128, 4, D2], bf16)

        def load_weights():
            nc.sync.dma_start(Wf_sb[:, 0], d_Wf[:, 0])
            nc.sync.dma_start(Wf_sb[:, 1], d_Wf[:, 1])
            nc.sync.dma_start(W1h_sb[:], d_W1h[:])
            for k in range(4):
                nc.sync.dma_start(W2_sb[:, k], d_W2[:, k])

        blocks = [(s, b) for s in range(n_scenes) for b in range(NB)]
        state = {}   # per-scene tiles
        mlp_q = []   # software pipeline: deferred MLP stage

        def scene_setup(s):
            tw = tw_pool.tile([128, P * P], bf16, tag="tw")
            nc.sync.dma_start(tw[64:80, :], d_tw[s])
            nc.sync.dma_start(tw[96:112, :], d_tw[s])
            # replicate on-chip (a broadcast DMA would emit one packet per
            # repeat per partition - hundreds of tiny descriptors)
            obs_c = scene_pool.tile([P, 16], bf16, tag="obs_c")
            nc.sync.dma_start(obs_c[:], d_obs[s * P:(s + 1) * P, :])
            obs_rep = scene_pool.tile([P, 128], bf16, tag="obs_rep")
            nc.vector.tensor_copy(
                obs_rep[:].rearrange("p (r g) -> p r g", r=8),
                obs_c[:].unsqueeze(1).broadcast_to([P, 8, 16]))
            # h broadcast over i: hj_fm[:, ii*P + j] = h_fm[s, :, j]
            h_c = scene_pool.tile([64, P], bf16, tag="h_c")
            nc.sync.dma_start(h_c[:], d_h[s])
            hj_fm = scene_pool.tile([64, NBLK], bf16, tag="hj_fm")
            nc.vector.tensor_copy(
                hj_fm[:].rearrange("p (r j) -> p r j", r=IB),
                h_c[:].unsqueeze(1).broadcast_to([64, IB, P]))
            pooled = scene_pool.tile([128, 4, 2 * P], f32, tag="pooled")
            state[s] = dict(tw=tw, obs_rep=obs_rep, hj_fm=hj_fm, pooled=pooled)

        def prep(s, b):
            st = state[s]
            c0 = b * NBLK
            rel_ps = pp.tile([128, NBLK], f32, tag="pp")
            nc.tensor.matmul(rel_ps[:], st["obs_rep"][:],
                             Dm_sb[:, c0:c0 + NBLK], start=True, stop=True,
                             tile_position=(0, 0))
            tw0_ps = pp.tile([128, NBLK], f32, tag="pp")
            nc.tensor.matmul(tw0_ps[:], Rb_sb[64:80, :],
                             st["tw"][64:80, c0:c0 + NBLK], start=True,
                             stop=True, tile_position=(64, 0))
            tw1_ps = pp.tile([128, NBLK], f32, tag="pp")
            nc.tensor.matmul(tw1_ps[:], Rb_sb[96:112, :],
                             st["tw"][96:112, c0:c0 + NBLK], start=True,
                             stop=True, tile_position=(96, 0))
            rel_sb = blk_pool.tile([128, NBLK], f32, tag="rel_sb")
            nc.vector.tensor_copy(rel_sb[:], rel_ps[:])
            rel2_0 = blk_pool.tile([128, NBLK], bf16, tag="rel2_0")
            nc.vector.tensor_tensor(rel2_0[:], tw0_ps[:], rel_sb[:], AluOpType.mult)
            rel2_1 = blk_pool.tile([128, NBLK], bf16, tag="rel2_1")
            nc.vector.tensor_tensor(rel2_1[:], tw1_ps[:], rel_sb[:], AluOpType.mult)
            return dict(rel2_0=rel2_0, rel2_1=rel2_1, s=s, b=b)

        def mlp1(job):
            s, b = job["s"], job["b"]
            st = state[s]
            r20 = job["rel2_0"][:]
            r21 = job["rel2_1"][:]
            x1 = blk_pool.tile([128, 4, NBLK], bf16, tag="x1")
            for m in range(4):
                p1t = p1.tile([128, NBLK], f32, tag="p1")
                nc.tensor.matmul(p1t[:], Wf_sb[:, 0, m * 128:(m + 1) * 128],
                                 r20, start=True, stop=False)
                nc.tensor.matmul(p1t[:], Wf_sb[:, 1, m * 128:(m + 1) * 128],
                                 r21, start=False, stop=False)
                nc.tensor.matmul(p1t[:], W1h_sb[:, m * 128:(m + 1) * 128],
                                 st["hj_fm"][:], start=False, stop=True)
                nc.scalar.activation(x1[:, m, :], p1t[:], AF.Relu,
                                     bias=b1_sb[:, m:m + 1])
            job["x1"] = x1

        def mlp2(job):
            s, b = job["s"], job["b"]
            st = state[s]
            x1 = job["x1"]
            last = b == NB - 1
            for mm in range(8):
                p2t = p2.tile([128, NBLK], f32, tag="p2")
                for k in range(4):
                    nc.tensor.matmul(
                        p2t[:], W2_sb[:, k, mm * 128:(mm + 1) * 128],
                        x1[:, k, :], start=(k == 0), stop=(k == 3))
                dst = st["pooled"][:, mm // 2,
                                   (mm % 2) * P + b * IB:(mm % 2) * P + (b + 1) * IB]
                nc.vector.tensor_reduce(
                    dst, p2t[:].rearrange("p (i j) -> p i j", i=IB),
                    axis=AX.X, op=AluOpType.max)
                # one-group delay so PE doesn't stall on the pair's
                # reduce -> transpose chain (no delay on the final scene,
                # where it would only stretch the kernel tail)
                if last and mm % 2 == 1:
                    if s == n_scenes - 1:
                        finish_pair(s, st, mm // 2)
                    elif mm >= 3:
                        finish_pair(s, st, (mm - 3) // 2)
            if last:
                if s != n_scenes - 1:
                    finish_pair(s, st, 2)
                    finish_pair(s, st, 3)
                state.pop(s)

        def finish_pair(s, st, pi):
            """Scene output for m-tile pair pi: bias+relu post-pool,
            transpose to row-major, stage to SBUF, DMA out."""
            pooled = st["pooled"]
            for half in range(2):
                mm = 2 * pi + half
                sl = pooled[:, pi, half * P:(half + 1) * P]
                nc.vector.scalar_tensor_tensor(
                    sl, sl, b2_sb[:, mm:mm + 1], zero_sb[:],
                    op0=AluOpType.add, op1=AluOpType.max)
            tps = p1.tile([128, NBLK], f32, tag="p1")
            nc.tensor.transpose(tps[:2 * P, :128], pooled[:, pi, :], id_sb[:])
            ot = scene_pool.tile([2 * P, 128], f32, tag="ot")
            nc.vector.tensor_copy(ot[:], tps[:2 * P, :128])
            nc.sync.dma_start(
                d_out[s * P:(s + 1) * P, (2 * pi) * 128:(2 * pi + 1) * 128],
                ot[:P, :])
            nc.sync.dma_start(
                d_out[s * P:(s + 1) * P, (2 * pi + 1) * 128:(2 * pi + 2) * 128],
                ot[P:2 * P, :])

        # two-deep software pipeline on PE:
        #   ... prep(i)  mlp1(i-1)  mlp2(i-2) ...
        # so x1 is ready a full block before MLP2 consumes it and PSUM
        # slot recycling has a block of slack; scene data is prefetched
        # one block before the scene starts
        scene_setup(0)
        for idx, (s, b) in enumerate(blocks):
            if b == NB - 2 and s + 1 < n_scenes:
                scene_setup(s + 1)
            if idx == 0:
                load_weights()
            mlp_q.append(prep(s, b))
            if len(mlp_q) > 1:
                mlp1(mlp_q[-2])
            if len(mlp_q) > 2:
                mlp2(mlp_q.pop(0))
        mlp1(mlp_q[-1])
        mlp2(mlp_q.pop(0))
        mlp2(mlp_q.pop(0))

    nc.compile()
    return nc


def _host_inputs(h_states, traj, traj_weight, consts, n_scenes=SC):
    """Slice + lay out per-core input maps (matmul operands cast to bf16)."""
    import ml_dtypes
    bf = ml_dtypes.bfloat16
    h_states = np.asarray(h_states, np.float32)
    traj = np.asarray(traj, np.float32)
    traj_weight = np.asarray(traj_weight, np.float32)

    obs_full = np.ascontiguousarray(
        traj[:T].transpose(1, 0, 2).reshape(B, 2 * T))          # (B,16) g=t*2+c
    h_full = h_states.reshape(S, P, H)

    consts = dict(consts)
    for k in ("Wf_sb", "W1h_sb", "W2_sb", "Dm", "R0", "R1"):
        consts[k] = consts[k].astype(bf)

    in_maps = []
    for core in range(NCORES):
        s0 = core * n_scenes
        sl = slice(s0, s0 + n_scenes)
        twT = np.ascontiguousarray(
            traj_weight[sl].transpose(0, 2, 3, 1).reshape(n_scenes, 16, P * P)
        ).astype(bf)
        h_fm = np.ascontiguousarray(h_full[sl].transpose(0, 2, 1)).astype(bf)
        obs_rm = np.ascontiguousarray(
            obs_full[s0 * P:(s0 + n_scenes) * P]).astype(bf)
        m = dict(obs_rm=obs_rm, twT=twT, h_fm=h_fm)
        m.update(consts)
        in_maps.append(m)
    return in_maps


def kernel(h_states, seq_start_end, end_pos, traj, traj_weight,
           mlp_pre_pool_dim_0, W_se, b_se, W1, b1, W2, b2):
    import sys
    if '/opt/trn_rl_repo' not in sys.path:
        sys.path.insert(0, '/opt/trn_rl_repo')
    from concourse.bass_utils import run_bass_kernel_spmd

    consts = _host_constants(W_se, W1, W2, b1, b2)
    in_maps = _host_inputs(h_states, traj, traj_weight, consts)
    nc = build_program(SC)
    res = run_bass_kernel_spmd(nc, in_maps, list(range(NCORES)))
    out = np.concatenate([res.results[i]["out"] for i in range(NCORES)], axis=0)
    return out.astype(np.float32)



# revision 5
# speedup vs baseline: 1.0848x; 1.0848x over previous
# Trainium2 Bass kernel for nn_Decoder_14568529068506 (gnn_message_passing).
#
# Reference computation (per scene s of 32, P=48 peds):
#   rel[i,j]  = obs[j] - obs[i]                  (P,P,2T)   2T=16
#   emb       = rel @ W_se.T                     (P,P,512)
#   emb      *= tile(traj_weight[s])             (P,P,512)
#   x         = concat([emb, h[j]], -1)          (P,P,576)
#   x1        = relu(x @ W1.T + b1)              (P,P,512)
#   x2        = relu(x1 @ W2.T + b2)             (P,P,1024)
#   out[s,i]  = max_j x2[i,j]                    (P,1024)
#
# Kernel restructuring (validated exactly in fp32 numpy):
#  * The tiled traj_weight multiply + spatial embedding + W1 are fused:
#      out1[d,row] = sum_{(ct,g)} Wf[d,(ct,g)] * tw[row,ct] * rel[row,g]
#    with Wf[d, ct*16+g] = sum_{k%2==c} W1[d, t*64+k] * W_se[t*64+k, g].
#    So MLP1 contracts over 256 "rel2" features (+64 h features) instead
#    of 576, and the (P,P,512) embedding is never materialized.
#  * rel2 = tw_rep * rel_rep feature-major on 128 partitions:
#      rel_rep = obs_rep.T @ D   (D = +-1 pairwise difference matrix, PE)
#      tw_rep  = 16x partition-replicated tw, built by a broadcast DMA
#      (reads each tw row 16x from DRAM; large 2.3KB packets)
#    so prep costs one matmul + two vector multiplies per row block.
#  * The h-state part of MLP1 rides as a third K=64 accumulation matmul
#    whose rhs (h broadcast over i) is built once per scene.
#  * relu/bias commute with max-pool: MLP2 PSUM outputs are max-pooled
#    directly; bias+relu run post-pool on the scalar engine.
#  * Output is written TRANSPOSED ([1024, P] per scene) straight from the
#    pooled tile via DMA; the host transposes back. This removes the PE
#    transposes and vector staging copies of the previous version.
#  * Matmuls run in bf16 (1 col/cycle @2.4GHz; LDWEIGHTS ~97ns hides under
#    the 160ns N=384 stream). PSUM accumulation stays fp32.
#  * PSUM: 3 single-tag pools (rel 2 banks, mlp1 3, mlp2 3) so slot
#    recycling never serializes PE on a vector-engine consumer.
#
# Sharding: scenes are data-parallel across the 8 cores (4 scenes each);
# weights replicated; per-core outputs concatenated on the host.

import numpy as np

S, P, T, E, H = 32, 48, 8, 64, 64
D1, D2 = 512, 1024
B = S * P
NCORES = 8
SC = S // NCORES          # scenes per core
NB = 6                    # row blocks per scene
NBLK = P * P // NB        # 384 columns (pairs) per block = 8 i-groups x 48 j
IB = NBLK // P            # i-groups per block (8)


def _host_constants(W_se, W1, W2, b1, b2):
    """Precompute fused weights + structural constant matrices (fp32)."""
    W_se = np.asarray(W_se, np.float32)
    W1 = np.asarray(W1, np.float32)
    W2 = np.asarray(W2, np.float32)
    b1 = np.asarray(b1, np.float32)
    b2 = np.asarray(b2, np.float32)

    W1e, W1h = W1[:, :512], W1[:, 512:]
    Wf = np.zeros((D1, 256), np.float32)
    for c in range(2):
        for t in range(T):
            ct = c * 8 + t
            f = t * 64 + np.arange(c, 64, 2)
            Wf[:, ct * 16:(ct + 1) * 16] = W1e[:, f] @ W_se[f, :]

    Dm = np.zeros((P, P * P), np.float32)
    ii, jj = np.meshgrid(np.arange(P), np.arange(P), indexing="ij")
    rows = (ii * P + jj).ravel()
    np.add.at(Dm, (jj.ravel(), rows), 1.0)
    np.add.at(Dm, (ii.ravel(), rows), -1.0)

    # lhsT tile layouts: [128, kTiles, M] so DMAs are contiguous
    Wf_sb = np.ascontiguousarray(Wf.T.reshape(2, 128, D1).transpose(1, 0, 2))
    W1h_sb = np.ascontiguousarray(W1h.T)                     # (64, 512)
    W2_sb = np.ascontiguousarray(W2.T.reshape(4, 128, D2).transpose(1, 0, 2))
    b1_sb = np.ascontiguousarray(b1.reshape(4, 128).T)       # (128, 4)
    b2_sb = np.ascontiguousarray(b2.reshape(8, 128).T)       # (128, 8)
    return dict(Wf_sb=Wf_sb, W1h_sb=W1h_sb, W2_sb=W2_sb, b1_sb=b1_sb,
                b2_sb=b2_sb, Dm=Dm)


def build_program(n_scenes=SC):
    """Emit the per-core Bass/Tile program. Returns the compiled Bacc."""
    from contextlib import ExitStack
    import concourse.bacc as bacc
    import concourse.tile as tile
    from concourse import mybir
    from concourse.alu_op_type import AluOpType

    f32 = mybir.dt.float32
    bf16 = mybir.dt.bfloat16
    AF = mybir.ActivationFunctionType
    AX = mybir.AxisListType

    nc = bacc.Bacc("TRN2", target_bir_lowering=False, debug=False)

    # ---- DRAM parameters -------------------------------------------------
    d_obs = nc.dram_tensor("obs_rm", [n_scenes * P, 16], bf16, kind="ExternalInput")
    d_tw0 = nc.dram_tensor("tw0r", [n_scenes, 128, P * P], bf16, kind="ExternalInput")
    d_tw1 = nc.dram_tensor("tw1r", [n_scenes, 128, P * P], bf16, kind="ExternalInput")
    d_h = nc.dram_tensor("h_fm", [n_scenes, 64, P], bf16, kind="ExternalInput")
    d_Dm = nc.dram_tensor("Dm", [P, P * P], bf16, kind="ExternalInput")
    d_Wf = nc.dram_tensor("Wf_sb", [128, 2, D1], bf16, kind="ExternalInput")
    d_W1h = nc.dram_tensor("W1h_sb", [64, D1], bf16, kind="ExternalInput")
    d_W2 = nc.dram_tensor("W2_sb", [128, 4, D2], bf16, kind="ExternalInput")
    d_b1 = nc.dram_tensor("b1_sb", [128, 4], f32, kind="ExternalInput")
    d_b2 = nc.dram_tensor("b2_sb", [128, 8], f32, kind="ExternalInput")
    # transposed output: out[d2, scene*P + i]; host transposes back
    d_out = nc.dram_tensor("out", [D2, n_scenes * P], f32, kind="ExternalOutput")

    with ExitStack() as ctx:
        tc = ctx.enter_context(tile.TileContext(nc))
        consts = ctx.enter_context(tc.tile_pool(name="consts", bufs=1))
        scene_pool = ctx.enter_context(tc.tile_pool(name="scene", bufs=2))
        blk_pool = ctx.enter_context(tc.tile_pool(name="blk", bufs=3))
        pp = ctx.enter_context(tc.tile_pool(name="pp", bufs=2, space="PSUM"))
        p1 = ctx.enter_context(tc.tile_pool(name="p1", bufs=3, space="PSUM"))
        p2 = ctx.enter_context(tc.tile_pool(name="p2", bufs=3, space="PSUM"))

        Dm_sb = consts.tile([P, P * P], bf16)
        b1_sb = consts.tile([128, 4], f32)
        b2_sb = consts.tile([128, 8], f32)
        Wf_sb = consts.tile([128, 2, D1], bf16)
        W1h_sb = consts.tile([64, D1], bf16)
        W2_sb = consts.tile([128, 4, D2], bf16)

        def load_weights():
            nc.sync.dma_start(Dm_sb[:], d_Dm[:])
            nc.sync.dma_start(Wf_sb[:, 0], d_Wf[:, 0])
            nc.sync.dma_start(Wf_sb[:, 1], d_Wf[:, 1])
            nc.sync.dma_start(W1h_sb[:], d_W1h[:])
            nc.sync.dma_start(b1_sb[:], d_b1[:])
            for k in range(4):
                nc.sync.dma_start(W2_sb[:, k], d_W2[:, k])
            nc.sync.dma_start(b2_sb[:], d_b2[:])

        blocks = [(s, b) for s in range(n_scenes) for b in range(NB)]
        state = {}   # per-scene tiles
        mlp_q = []   # software pipeline: deferred MLP stages

        def scene_setup(s):
            # tw arrives pre-replicated 16x along partitions from the host;
            # two plain contiguous [128, 4.6KB] loads per scene
            tw0 = scene_pool.tile([128, P * P], bf16, tag="tw0")
            tw1 = scene_pool.tile([128, P * P], bf16, tag="tw1")
            nc.sync.dma_start(tw0[:], d_tw0[s])
            nc.sync.dma_start(tw1[:], d_tw1[s])
            obs_c = scene_pool.tile([P, 16], bf16, tag="obs_c")
            nc.sync.dma_start(obs_c[:], d_obs[s * P:(s + 1) * P, :])
            obs_rep = scene_pool.tile([P, 128], bf16, tag="obs_rep")
            nc.vector.tensor_copy(
                obs_rep[:].rearrange("p (r g) -> p r g", r=8),
                obs_c[:].unsqueeze(1).broadcast_to([P, 8, 16]))
            # h broadcast over i: hj_fm[:, ii*P + j] = h_fm[s, :, j]
            h_c = scene_pool.tile([64, P], bf16, tag="h_c")
            nc.sync.dma_start(h_c[:], d_h[s])
            hj_fm = scene_pool.tile([64, NBLK], bf16, tag="hj_fm")
            nc.vector.tensor_copy(
                hj_fm[:].rearrange("p (r j) -> p r j", r=IB),
                h_c[:].unsqueeze(1).broadcast_to([64, IB, P]))
            pooled = scene_pool.tile([128, 8, P], f32, tag="pooled")
            state[s] = dict(tw0=tw0, tw1=tw1, obs_rep=obs_rep, hj_fm=hj_fm,
                            pooled=pooled)

        def prep(s, b):
            st = state[s]
            c0 = b * NBLK
            rel_ps = pp.tile([128, NBLK], f32, tag="pp")
            nc.tensor.matmul(rel_ps[:], st["obs_rep"][:],
                             Dm_sb[:, c0:c0 + NBLK], start=True, stop=True,
                             tile_position=(0, 0))
            rel2_0 = blk_pool.tile([128, NBLK], bf16, tag="rel2_0")
            nc.vector.tensor_tensor(rel2_0[:], st["tw0"][:, c0:c0 + NBLK],
                                    rel_ps[:], AluOpType.mult)
            rel2_1 = blk_pool.tile([128, NBLK], bf16, tag="rel2_1")
            nc.vector.tensor_tensor(rel2_1[:], st["tw1"][:, c0:c0 + NBLK],
                                    rel_ps[:], AluOpType.mult)
            return dict(rel2_0=rel2_0, rel2_1=rel2_1, s=s, b=b)

        def mlp1(job):
            st = state[job["s"]]
            r20 = job["rel2_0"][:]
            r21 = job["rel2_1"][:]
            x1 = blk_pool.tile([128, 4, NBLK], bf16, tag="x1")
            for m in range(4):
                p1t = p1.tile([128, NBLK], f32, tag="p1")
                nc.tensor.matmul(p1t[:], Wf_sb[:, 0, m * 128:(m + 1) * 128],
                                 r20, start=True, stop=False)
                nc.tensor.matmul(p1t[:], Wf_sb[:, 1, m * 128:(m + 1) * 128],
                                 r21, start=False, stop=False)
                nc.tensor.matmul(p1t[:], W1h_sb[:, m * 128:(m + 1) * 128],
                                 st["hj_fm"][:], start=False, stop=True)
                nc.scalar.activation(x1[:, m, :], p1t[:], AF.Relu,
                                     bias=b1_sb[:, m:m + 1])
            job["x1"] = x1

        def mlp2(job):
            s, b = job["s"], job["b"]
            st = state[s]
            x1 = job["x1"]
            pooled = st["pooled"]
            last = b == NB - 1
            for mm in range(8):
                p2t = p2.tile([128, NBLK], f32, tag="p2")
                for k in range(4):
                    nc.tensor.matmul(
                        p2t[:], W2_sb[:, k, mm * 128:(mm + 1) * 128],
                        x1[:, k, :], start=(k == 0), stop=(k == 3))
                nc.vector.tensor_reduce(
                    pooled[:, mm, b * IB:(b + 1) * IB],
                    p2t[:].rearrange("p (i j) -> p i j", i=IB),
                    axis=AX.X, op=AluOpType.max)
                if last:
                    # scene output for m-tile mm: bias+relu post-pool on the
                    # scalar engine, then DMA the [128, P] slice transposed
                    nc.scalar.activation(pooled[:, mm, :], pooled[:, mm, :],
                                         AF.Relu, bias=b2_sb[:, mm:mm + 1])
                    nc.sync.dma_start(
                        d_out[mm * 128:(mm + 1) * 128, s * P:(s + 1) * P],
                        pooled[:, mm, :])
            if last:
                state.pop(s)

        # two-deep software pipeline on PE:
        #   ... prep(i)  mlp1(i-1)  mlp2(i-2) ...
        # so x1 is ready a full block before MLP2 consumes it; scene data is
        # prefetched two blocks before the scene starts; weights stream in
        # behind scene 0's data.
        scene_setup(0)
        load_weights()
        for s, b in blocks:
            if b == NB - 2 and s + 1 < n_scenes:
                scene_setup(s + 1)
            mlp_q.append(prep(s, b))
            if len(mlp_q) > 1:
                mlp1(mlp_q[-2])
            if len(mlp_q) > 2:
                mlp2(mlp_q.pop(0))
        mlp1(mlp_q[-1])
        mlp2(mlp_q.pop(0))
        mlp2(mlp_q.pop(0))

    nc.compile()
    return nc


def _host_inputs(h_states, traj, traj_weight, consts, n_scenes=SC):
    """Slice + lay out per-core input maps (matmul operands cast to bf16)."""
    import ml_dtypes
    bf = ml_dtypes.bfloat16
    h_states = np.asarray(h_states, np.float32)
    traj = np.asarray(traj, np.float32)
    traj_weight = np.asarray(traj_weight, np.float32)

    obs_full = np.ascontiguousarray(
        traj[:T].transpose(1, 0, 2).reshape(B, 2 * T))          # (B,16) g=t*2+c
    h_full = h_states.reshape(S, P, H)

    consts = dict(consts)
    for k in ("Wf_sb", "W1h_sb", "W2_sb", "Dm"):
        consts[k] = consts[k].astype(bf)

    in_maps = []
    for core in range(NCORES):
        s0 = core * n_scenes
        sl = slice(s0, s0 + n_scenes)
        twT = np.ascontiguousarray(
            traj_weight[sl].transpose(0, 2, 3, 1).reshape(n_scenes, 16, P * P))
        # pre-replicate each tw row 16x along partitions (feature-major)
        tw0r = np.ascontiguousarray(np.repeat(twT[:, 0:8], 16, axis=1)).astype(bf)
        tw1r = np.ascontiguousarray(np.repeat(twT[:, 8:16], 16, axis=1)).astype(bf)
        h_fm = np.ascontiguousarray(h_full[sl].transpose(0, 2, 1)).astype(bf)
        obs_rm = np.ascontiguousarray(
            obs_full[s0 * P:(s0 + n_scenes) * P]).astype(bf)
        m = dict(obs_rm=obs_rm, tw0r=tw0r, tw1r=tw1r, h_fm=h_fm)
        m.update(consts)
        in_maps.append(m)
    return in_maps


def kernel(h_states, seq_start_end, end_pos, traj, traj_weight,
           mlp_pre_pool_dim_0, W_se, b_se, W1, b1, W2, b2):
    import sys
    if '/opt/trn_rl_repo' not in sys.path:
        sys.path.insert(0, '/opt/trn_rl_repo')
    from concourse.bass_utils import run_bass_kernel_spmd

    consts = _host_constants(W_se, W1, W2, b1, b2)
    in_maps = _host_inputs(h_states, traj, traj_weight, consts)
    nc = build_program(SC)
    res = run_bass_kernel_spmd(nc, in_maps, list(range(NCORES)))
    out = np.concatenate(
        [res.results[i]["out"].T for i in range(NCORES)], axis=0)
    return np.ascontiguousarray(out).astype(np.float32)


# revision 12
# speedup vs baseline: 1.0913x; 1.0061x over previous
# Trainium2 Bass kernel for nn_Decoder_14568529068506 (gnn_message_passing).
#
# Reference computation (per scene s of 32, P=48 peds):
#   rel[i,j]  = obs[j] - obs[i]                  (P,P,2T)   2T=16
#   emb       = rel @ W_se.T                     (P,P,512)
#   emb      *= tile(traj_weight[s])             (P,P,512)
#   x         = concat([emb, h[j]], -1)          (P,P,576)
#   x1        = relu(x @ W1.T + b1)              (P,P,512)
#   x2        = relu(x1 @ W2.T + b2)             (P,P,1024)
#   out[s,i]  = max_j x2[i,j]                    (P,1024)
#
# Kernel restructuring (validated exactly in fp32 numpy):
#  * The tiled traj_weight multiply + spatial embedding + W1 are fused:
#      out1[d,row] = sum_{(ct,g)} Wf[d,(ct,g)] * tw[row,ct] * rel[row,g]
#    with Wf[d, ct*16+g] = sum_{k%2==c} W1[d, t*64+k] * W_se[t*64+k, g].
#    So MLP1 contracts over 256 "rel2" features (+64 h features) instead
#    of 576, and the (P,P,512) embedding is never materialized.
#  * rel2 = tw_rep * rel_rep feature-major on 128 partitions:
#      rel_rep = obs_rep.T @ D   (D = +-1 pairwise difference matrix, PE)
#      tw_rep  = 16x partition-replicated tw, built by a broadcast DMA
#      (reads each tw row 16x from DRAM; large 2.3KB packets)
#    so prep costs one matmul + two vector multiplies per row block.
#  * The h-state part of MLP1 rides as a third K=64 accumulation matmul
#    whose rhs (h broadcast over i) is built once per scene.
#  * relu/bias commute with max-pool: MLP2 PSUM outputs are max-pooled
#    directly; bias+relu run post-pool on the scalar engine.
#  * Output is written TRANSPOSED ([1024, P] per scene) straight from the
#    pooled tile via DMA; the host transposes back. This removes the PE
#    transposes and vector staging copies of the previous version.
#  * Matmuls run in bf16 (1 col/cycle @2.4GHz; LDWEIGHTS ~97ns hides under
#    the 160ns N=384 stream). PSUM accumulation stays fp32.
#  * PSUM: 3 single-tag pools (rel 2 banks, mlp1 3, mlp2 3) so slot
#    recycling never serializes PE on a vector-engine consumer.
#
# Sharding: scenes are data-parallel across the 8 cores (4 scenes each);
# weights replicated; per-core outputs concatenated on the host.

import numpy as np

S, P, T, E, H = 32, 48, 8, 64, 64
D1, D2 = 512, 1024
B = S * P
NCORES = 8
SC = S // NCORES          # scenes per core
NB = 6                    # row blocks per scene
NBLK = P * P // NB        # 384 columns (pairs) per block = 8 i-groups x 48 j
IB = NBLK // P            # i-groups per block (8)


def _host_constants(W_se, W1, W2, b1, b2):
    """Precompute fused weights + structural constant matrices (fp32)."""
    W_se = np.asarray(W_se, np.float32)
    W1 = np.asarray(W1, np.float32)
    W2 = np.asarray(W2, np.float32)
    b1 = np.asarray(b1, np.float32)
    b2 = np.asarray(b2, np.float32)

    W1e, W1h = W1[:, :512], W1[:, 512:]
    Wf = np.zeros((D1, 256), np.float32)
    for c in range(2):
        for t in range(T):
            ct = c * 8 + t
            f = t * 64 + np.arange(c, 64, 2)
            Wf[:, ct * 16:(ct + 1) * 16] = W1e[:, f] @ W_se[f, :]

    Dm = np.zeros((P, P * P), np.float32)
    ii, jj = np.meshgrid(np.arange(P), np.arange(P), indexing="ij")
    rows = (ii * P + jj).ravel()
    np.add.at(Dm, (jj.ravel(), rows), 1.0)
    np.add.at(Dm, (ii.ravel(), rows), -1.0)

    # lhsT tile layouts: [128, kTiles, M] so DMAs are contiguous
    Wf_sb = np.ascontiguousarray(Wf.T.reshape(2, 128, D1).transpose(1, 0, 2))
    W1h_sb = np.ascontiguousarray(W1h.T)                     # (64, 512)
    W2_sb = np.ascontiguousarray(W2.T.reshape(4, 128, D2).transpose(1, 0, 2))
    b1_sb = np.ascontiguousarray(b1.reshape(4, 128).T)       # (128, 4)
    b2_sb = np.ascontiguousarray(b2.reshape(8, 128).T)       # (128, 8)
    return dict(Wf_sb=Wf_sb, W1h_sb=W1h_sb, W2_sb=W2_sb, b1_sb=b1_sb,
                b2_sb=b2_sb, Dm=Dm)


def build_program(n_scenes=SC):
    """Emit the per-core Bass/Tile program. Returns the compiled Bacc."""
    from contextlib import ExitStack
    import concourse.bacc as bacc
    import concourse.tile as tile
    from concourse import mybir
    from concourse.alu_op_type import AluOpType

    f32 = mybir.dt.float32
    bf16 = mybir.dt.bfloat16
    AF = mybir.ActivationFunctionType
    AX = mybir.AxisListType

    nc = bacc.Bacc("TRN2", target_bir_lowering=False, debug=False)

    # ---- DRAM parameters -------------------------------------------------
    d_obs = nc.dram_tensor("obs_rm", [n_scenes * P, 16], bf16, kind="ExternalInput")
    d_tw0 = nc.dram_tensor("tw0r", [n_scenes, 128, P * P], bf16, kind="ExternalInput")
    d_tw1 = nc.dram_tensor("tw1r", [n_scenes, 128, P * P], bf16, kind="ExternalInput")
    d_h = nc.dram_tensor("h_fm", [n_scenes, 64, P], bf16, kind="ExternalInput")
    d_Dm = nc.dram_tensor("Dm", [P, P * P], bf16, kind="ExternalInput")
    d_Wf = nc.dram_tensor("Wf_sb", [128, 2, D1], bf16, kind="ExternalInput")
    d_W1h = nc.dram_tensor("W1h_sb", [64, D1], bf16, kind="ExternalInput")
    d_W2 = nc.dram_tensor("W2_sb", [128, 4, D2], bf16, kind="ExternalInput")
    d_b1 = nc.dram_tensor("b1_sb", [128, 4], f32, kind="ExternalInput")
    d_b2 = nc.dram_tensor("b2_sb", [128, 8], f32, kind="ExternalInput")
    # transposed output: out[d2, scene*P + i]; host transposes back
    d_out = nc.dram_tensor("out", [D2, n_scenes * P], f32, kind="ExternalOutput")

    with ExitStack() as ctx:
        tc = ctx.enter_context(tile.TileContext(nc))
        consts = ctx.enter_context(tc.tile_pool(name="consts", bufs=1))
        scene_pool = ctx.enter_context(tc.tile_pool(name="scene", bufs=2))
        blk_pool = ctx.enter_context(tc.tile_pool(name="blk", bufs=3))
        # PSUM: 1 + 3 + 2x2 banks. p2 tiles span two banks so one reduce
        # covers two MLP2 m-tiles (halves reduce count and PE stop-semaphore
        # updates, which cost ~100ns each on the PE pipeline).
        pp = ctx.enter_context(tc.tile_pool(name="pp", bufs=1, space="PSUM"))
        p1 = ctx.enter_context(tc.tile_pool(name="p1", bufs=3, space="PSUM"))
        p2 = ctx.enter_context(tc.tile_pool(name="p2", bufs=2, space="PSUM"))

        Dm_sb = consts.tile([P, P * P], bf16)
        b1_sb = consts.tile([128, 4], f32)
        b2_sb = consts.tile([128, 8], f32)
        Wf_sb = consts.tile([128, 2, D1], bf16)
        W1h_sb = consts.tile([64, D1], bf16)
        W2_sb = consts.tile([128, 4, D2], bf16)

        def load_weights():
            nc.sync.dma_start(Dm_sb[:], d_Dm[:])
            nc.sync.dma_start(Wf_sb[:, 0], d_Wf[:, 0])
            nc.sync.dma_start(Wf_sb[:, 1], d_Wf[:, 1])
            nc.sync.dma_start(W1h_sb[:], d_W1h[:])
            nc.sync.dma_start(b1_sb[:], d_b1[:])
            for k in range(4):
                nc.sync.dma_start(W2_sb[:, k], d_W2[:, k])
            nc.sync.dma_start(b2_sb[:], d_b2[:])

        blocks = [(s, b) for s in range(n_scenes) for b in range(NB)]
        state = {}   # per-scene tiles
        mlp_q = []   # software pipeline: deferred MLP stages

        def scene_setup(s, split=False):
            # tw arrives pre-replicated 16x along partitions from the host;
            # two plain contiguous [128, 4.6KB] loads per scene. For scene 0
            # only the first-half columns load now; the rest queues behind
            # the weights (blocks 0-2 touch cols < P*P/2 only).
            tw0 = scene_pool.tile([128, P * P], bf16, tag="tw0")
            tw1 = scene_pool.tile([128, P * P], bf16, tag="tw1")
            cend = P * P // 2 if split else P * P
            nc.sync.dma_start(tw0[:, :cend], d_tw0[s, :, :cend])
            nc.sync.dma_start(tw1[:, :cend], d_tw1[s, :, :cend])
            obs_c = scene_pool.tile([P, 16], bf16, tag="obs_c")
            nc.sync.dma_start(obs_c[:], d_obs[s * P:(s + 1) * P, :])
            obs_rep = scene_pool.tile([P, 128], bf16, tag="obs_rep")
            nc.vector.tensor_copy(
                obs_rep[:].rearrange("p (r g) -> p r g", r=8),
                obs_c[:].unsqueeze(1).broadcast_to([P, 8, 16]))
            # h broadcast over i: hj_fm[:, ii*P + j] = h_fm[s, :, j]
            h_c = scene_pool.tile([64, P], bf16, tag="h_c")
            nc.sync.dma_start(h_c[:], d_h[s])
            hj_fm = scene_pool.tile([64, NBLK], bf16, tag="hj_fm")
            nc.vector.tensor_copy(
                hj_fm[:].rearrange("p (r j) -> p r j", r=IB),
                h_c[:].unsqueeze(1).broadcast_to([64, IB, P]))
            pooled = scene_pool.tile([128, 8, P], f32, tag="pooled")
            state[s] = dict(tw0=tw0, tw1=tw1, obs_rep=obs_rep, hj_fm=hj_fm,
                            pooled=pooled)
            return tw0, tw1

        def prep(s, b):
            st = state[s]
            c0 = b * NBLK
            rel_ps = pp.tile([128, NBLK], f32, tag="pp")
            nc.tensor.matmul(rel_ps[:], st["obs_rep"][:],
                             Dm_sb[:, c0:c0 + NBLK], start=True, stop=True,
                             tile_position=(0, 0))
            rel2_0 = blk_pool.tile([128, NBLK], bf16, tag="rel2_0")
            nc.vector.tensor_tensor(rel2_0[:], st["tw0"][:, c0:c0 + NBLK],
                                    rel_ps[:], AluOpType.mult)
            rel2_1 = blk_pool.tile([128, NBLK], bf16, tag="rel2_1")
            nc.vector.tensor_tensor(rel2_1[:], st["tw1"][:, c0:c0 + NBLK],
                                    rel_ps[:], AluOpType.mult)
            return dict(rel2_0=rel2_0, rel2_1=rel2_1, s=s, b=b)

        def mlp1(job):
            st = state[job["s"]]
            r20 = job["rel2_0"][:]
            r21 = job["rel2_1"][:]
            x1 = blk_pool.tile([128, 4, NBLK], bf16, tag="x1")
            for m in range(4):
                p1t = p1.tile([128, NBLK], f32, tag="p1")
                nc.tensor.matmul(p1t[:], Wf_sb[:, 0, m * 128:(m + 1) * 128],
                                 r20, start=True, stop=False)
                nc.tensor.matmul(p1t[:], Wf_sb[:, 1, m * 128:(m + 1) * 128],
                                 r21, start=False, stop=False)
                nc.tensor.matmul(p1t[:], W1h_sb[:, m * 128:(m + 1) * 128],
                                 st["hj_fm"][:], start=False, stop=True)
                nc.scalar.activation(x1[:, m, :], p1t[:], AF.Relu,
                                     bias=b1_sb[:, m:m + 1])
            job["x1"] = x1

        def mlp2_mpair(job, mp, p2t=None):
            """MLP2 m-tiles 2*mp, 2*mp+1 of one block into a 2-bank PSUM
            pair tile, then one reduce covering both. Returns the pair tile
            for reuse by a sibling block."""
            s, b = job["s"], job["b"]
            x1 = job["x1"]
            if p2t is None:
                p2t = p2.tile([128, 2, 512], f32, tag="p2")
            for half in range(2):
                mm = 2 * mp + half
                for k in range(4):
                    nc.tensor.matmul(
                        p2t[:, half, :NBLK], W2_sb[:, k, mm * 128:(mm + 1) * 128],
                        x1[:, k, :], start=(k == 0), stop=(k == 3))
            nc.vector.tensor_reduce(
                state[s]["pooled"][:, 2 * mp:2 * mp + 2, b * IB:(b + 1) * IB],
                p2t[:, :, :NBLK].rearrange("p h (i j) -> p h i j", j=P),
                axis=AX.X, op=AluOpType.max)
            return p2t

        def finish_m(s, mm):
            # scene output for m-tile mm: bias+relu post-pool on the scalar
            # engine, then DMA the [128, P] slice (output is transposed)
            pooled = state[s]["pooled"]
            nc.scalar.activation(pooled[:, mm, :], pooled[:, mm, :],
                                 AF.Relu, bias=b2_sb[:, mm:mm + 1])
            nc.sync.dma_start(
                d_out[mm * 128:(mm + 1) * 128, s * P:(s + 1) * P],
                pooled[:, mm, :])

        def mlp2(job):
            s, b = job["s"], job["b"]
            last = b == NB - 1
            for mp in range(4):
                mlp2_mpair(job, mp)
                if last:
                    finish_m(s, 2 * mp)
                    finish_m(s, 2 * mp + 1)
            if last:
                state.pop(s)

        def mlp2_final_pair(job_a, job_b):
            """Last two blocks of the final scene, m-pair interleaved so the
            vector-engine reduce queue keeps pace with PE and the kernel
            tail is one m-pair deep instead of two blocks deep."""
            s = job_b["s"]
            for mp in range(4):
                mlp2_mpair(job_a, mp)
                mlp2_mpair(job_b, mp)
                finish_m(s, 2 * mp)
                finish_m(s, 2 * mp + 1)
            state.pop(s)

        # two-deep software pipeline on PE:
        #   ... prep(i)  mlp1(i-1)  mlp2(i-2) ...
        # so x1 is ready a full block before MLP2 consumes it; scene data is
        # prefetched two blocks before the scene starts; weights stream in
        # behind scene 0's data.
        tw0_0, tw1_0 = scene_setup(0, split=True)
        load_weights()
        HPP = P * P // 2
        nc.sync.dma_start(tw0_0[:, HPP:], d_tw0[0, :, HPP:])
        nc.sync.dma_start(tw1_0[:, HPP:], d_tw1[0, :, HPP:])
        for s, b in blocks:
            if b == NB - 2 and s + 1 < n_scenes:
                scene_setup(s + 1)
            mlp_q.append(prep(s, b))
            if len(mlp_q) > 1:
                mlp1(mlp_q[-2])
            if len(mlp_q) > 2:
                mlp2(mlp_q.pop(0))
        mlp1(mlp_q[-1])
        mlp2_final_pair(mlp_q.pop(0), mlp_q.pop(0))

    nc.compile()
    return nc


def _host_inputs(h_states, traj, traj_weight, consts, n_scenes=SC):
    """Slice + lay out per-core input maps (matmul operands cast to bf16)."""
    import ml_dtypes
    bf = ml_dtypes.bfloat16
    h_states = np.asarray(h_states, np.float32)
    traj = np.asarray(traj, np.float32)
    traj_weight = np.asarray(traj_weight, np.float32)

    obs_full = np.ascontiguousarray(
        traj[:T].transpose(1, 0, 2).reshape(B, 2 * T))          # (B,16) g=t*2+c
    h_full = h_states.reshape(S, P, H)

    consts = dict(consts)
    for k in ("Wf_sb", "W1h_sb", "W2_sb", "Dm"):
        consts[k] = consts[k].astype(bf)

    in_maps = []
    for core in range(NCORES):
        s0 = core * n_scenes
        sl = slice(s0, s0 + n_scenes)
        twT = np.ascontiguousarray(
            traj_weight[sl].transpose(0, 2, 3, 1).reshape(n_scenes, 16, P * P))
        # pre-replicate each tw row 16x along partitions (feature-major)
        tw0r = np.ascontiguousarray(np.repeat(twT[:, 0:8], 16, axis=1)).astype(bf)
        tw1r = np.ascontiguousarray(np.repeat(twT[:, 8:16], 16, axis=1)).astype(bf)
        h_fm = np.ascontiguousarray(h_full[sl].transpose(0, 2, 1)).astype(bf)
        obs_rm = np.ascontiguousarray(
            obs_full[s0 * P:(s0 + n_scenes) * P]).astype(bf)
        m = dict(obs_rm=obs_rm, tw0r=tw0r, tw1r=tw1r, h_fm=h_fm)
        m.update(consts)
        in_maps.append(m)
    return in_maps


def kernel(h_states, seq_start_end, end_pos, traj, traj_weight,
           mlp_pre_pool_dim_0, W_se, b_se, W1, b1, W2, b2):
    import sys
    if '/opt/trn_rl_repo' not in sys.path:
        sys.path.insert(0, '/opt/trn_rl_repo')
    from concourse.bass_utils import run_bass_kernel_spmd

    consts = _host_constants(W_se, W1, W2, b1, b2)
    in_maps = _host_inputs(h_states, traj, traj_weight, consts)
    nc = build_program(SC)
    res = run_bass_kernel_spmd(nc, in_maps, list(range(NCORES)))
    out = np.concatenate(
        [res.results[i]["out"].T for i in range(NCORES)], axis=0)
    return np.ascontiguousarray(out).astype(np.float32)


# revision 20
# speedup vs baseline: 1.2093x; 1.1081x over previous
# Trainium2 Bass kernel for nn_Decoder_14568529068506 (gnn_message_passing).
#
# Reference computation (per scene s of 32, P=48 peds):
#   rel[i,j]  = obs[j] - obs[i]                  (P,P,2T)   2T=16
#   emb       = rel @ W_se.T                     (P,P,512)
#   emb      *= tile(traj_weight[s])             (P,P,512)
#   x         = concat([emb, h[j]], -1)          (P,P,576)
#   x1        = relu(x @ W1.T + b1)              (P,P,512)
#   x2        = relu(x1 @ W2.T + b2)             (P,P,1024)
#   out[s,i]  = max_j x2[i,j]                    (P,1024)
#
# Kernel restructuring (validated exactly in fp32 numpy):
#  * The tiled traj_weight multiply + spatial embedding + W1 are fused:
#      out1[d,row] = sum_{(ct,g)} Wf[d,(ct,g)] * tw[row,ct] * rel[row,g]
#    with Wf[d, ct*16+g] = sum_{k%2==c} W1[d, t*64+k] * W_se[t*64+k, g].
#    So MLP1 contracts over 256 "rel2" features (+64 h features) instead
#    of 576, and the (P,P,512) embedding is never materialized.
#  * rel2 = tw_rep * rel_rep feature-major on 128 partitions:
#      rel_rep = obs_rep.T @ D   (D = +-1 pairwise difference matrix, PE)
#      tw_rep  = 16x partition-replicated tw, built by a broadcast DMA
#      (reads each tw row 16x from DRAM; large 2.3KB packets)
#    so prep costs one matmul + two vector multiplies per row block.
#  * The h-state part of MLP1 rides as a third K=64 accumulation matmul
#    whose rhs (h broadcast over i) is built once per scene.
#  * relu/bias commute with max-pool: MLP2 PSUM outputs are max-pooled
#    directly; bias+relu run post-pool on the scalar engine.
#  * Output is written TRANSPOSED ([1024, P] per scene) straight from the
#    pooled tile via DMA; the host transposes back. This removes the PE
#    transposes and vector staging copies of the previous version.
#  * Matmuls run in bf16 (1 col/cycle @2.4GHz; LDWEIGHTS ~97ns hides under
#    the 160ns N=384 stream). PSUM accumulation stays fp32.
#  * PSUM: 3 single-tag pools (rel 2 banks, mlp1 3, mlp2 3) so slot
#    recycling never serializes PE on a vector-engine consumer.
#
# Sharding: scenes are data-parallel across the 8 cores (4 scenes each);
# weights replicated; per-core outputs concatenated on the host.

import numpy as np

S, P, T, E, H = 32, 48, 8, 64, 64
D1, D2 = 512, 1024
B = S * P
NCORES = 8
SC = S // NCORES          # scenes per core
NB = 6                    # row blocks per scene
NBLK = P * P // NB        # 384 columns (pairs) per block = 8 i-groups x 48 j
IB = NBLK // P            # i-groups per block (8)


def _host_constants(W_se, W1, W2, b1, b2):
    """Precompute fused weights + structural constant matrices (fp32)."""
    W_se = np.asarray(W_se, np.float32)
    W1 = np.asarray(W1, np.float32)
    W2 = np.asarray(W2, np.float32)
    b1 = np.asarray(b1, np.float32)
    b2 = np.asarray(b2, np.float32)

    W1e, W1h = W1[:, :512], W1[:, 512:]
    Wf = np.zeros((D1, 256), np.float32)
    for c in range(2):
        for t in range(T):
            ct = c * 8 + t
            f = t * 64 + np.arange(c, 64, 2)
            Wf[:, ct * 16:(ct + 1) * 16] = W1e[:, f] @ W_se[f, :]

    # Dm zero-padded to K=128: all matmuls use the full 128-row PE group
    # (switching PE row groups costs ~100ns each way on TRN2)
    Dm = np.zeros((128, P * P), np.float32)
    ii, jj = np.meshgrid(np.arange(P), np.arange(P), indexing="ij")
    rows = (ii * P + jj).ravel()
    np.add.at(Dm, (jj.ravel(), rows), 1.0)
    np.add.at(Dm, (ii.ravel(), rows), -1.0)

    # lhsT tile layouts: [128, kTiles, M] so DMAs are contiguous
    Wf_sb = np.ascontiguousarray(Wf.T.reshape(2, 128, D1).transpose(1, 0, 2))
    # W1h/2 stacked twice (rows exact in bf16): h rides K=128 with h
    # replicated to both partition halves
    W1h_sb = np.ascontiguousarray(np.vstack([W1h.T, W1h.T]) * 0.5)  # (128, 512)
    W2_sb = np.ascontiguousarray(W2.T.reshape(4, 128, D2).transpose(1, 0, 2))
    b1_sb = np.ascontiguousarray(b1.reshape(4, 128).T)       # (128, 4)
    b2_sb = np.ascontiguousarray(b2.reshape(8, 128).T)       # (128, 8)
    return dict(Wf_sb=Wf_sb, W1h_sb=W1h_sb, W2_sb=W2_sb, b1_sb=b1_sb,
                b2_sb=b2_sb, Dm=Dm)


def build_program(n_scenes=SC):
    """Emit the per-core Bass/Tile program. Returns the compiled Bacc."""
    from contextlib import ExitStack
    import concourse.bacc as bacc
    import concourse.tile as tile
    from concourse import mybir
    from concourse.alu_op_type import AluOpType

    f32 = mybir.dt.float32
    bf16 = mybir.dt.bfloat16
    AF = mybir.ActivationFunctionType
    AX = mybir.AxisListType

    nc = bacc.Bacc("TRN2", target_bir_lowering=False, debug=False)

    # ---- DRAM parameters -------------------------------------------------
    d_obs = nc.dram_tensor("obs_rm", [n_scenes * P, 16], bf16, kind="ExternalInput")
    d_tw0 = nc.dram_tensor("tw0r", [n_scenes, 128, P * P], bf16, kind="ExternalInput")
    d_tw1 = nc.dram_tensor("tw1r", [n_scenes, 128, P * P], bf16, kind="ExternalInput")
    d_h = nc.dram_tensor("h_fm", [n_scenes, 64, P], bf16, kind="ExternalInput")
    d_Dm = nc.dram_tensor("Dm", [128, P * P], bf16, kind="ExternalInput")
    d_Wf = nc.dram_tensor("Wf_sb", [128, 2, D1], bf16, kind="ExternalInput")
    d_W1h = nc.dram_tensor("W1h_sb", [128, D1], bf16, kind="ExternalInput")
    d_W2 = nc.dram_tensor("W2_sb", [128, 4, D2], bf16, kind="ExternalInput")
    d_b1 = nc.dram_tensor("b1_sb", [128, 4], f32, kind="ExternalInput")
    d_b2 = nc.dram_tensor("b2_sb", [128, 8], f32, kind="ExternalInput")
    # transposed output: out[d2, scene*P + i]; host transposes back
    d_out = nc.dram_tensor("out", [D2, n_scenes * P], f32, kind="ExternalOutput")

    with ExitStack() as ctx:
        tc = ctx.enter_context(tile.TileContext(nc))
        consts = ctx.enter_context(tc.tile_pool(name="consts", bufs=1))
        scene_pool = ctx.enter_context(tc.tile_pool(name="scene", bufs=2))
        blk_pool = ctx.enter_context(tc.tile_pool(name="blk", bufs=3))
        # PSUM: 1 + 3 + 2x2 banks. p2 tiles span two banks so one reduce
        # covers two MLP2 m-tiles (halves reduce count and PE stop-semaphore
        # updates, which cost ~100ns each on the PE pipeline).
        pp = ctx.enter_context(tc.tile_pool(name="pp", bufs=1, space="PSUM"))
        p1 = ctx.enter_context(tc.tile_pool(name="p1", bufs=3, space="PSUM"))
        p2 = ctx.enter_context(tc.tile_pool(name="p2", bufs=2, space="PSUM"))

        Dm_sb = consts.tile([128, P * P], bf16)
        b1_sb = consts.tile([128, 4], f32)
        b2_sb = consts.tile([128, 8], f32)
        Wf_sb = consts.tile([128, 2, D1], bf16)
        W1h_sb = consts.tile([128, D1], bf16)
        W2_sb = consts.tile([128, 4, D2], bf16)

        blocks = [(s, b) for s in range(n_scenes) for b in range(NB)]
        state = {}   # per-scene tiles
        mlp_q = []   # software pipeline: deferred MLP stages

        def scene_setup(s, split=False):
            # tw arrives pre-replicated 16x along partitions from the host;
            # two plain contiguous [128, 4.6KB] loads per scene. For scene 0
            # only the first-half columns load now; the rest queues behind
            # the weights (blocks 0-2 touch cols < P*P/2 only).
            tw0 = scene_pool.tile([128, P * P], bf16, tag="tw0")
            tw1 = scene_pool.tile([128, P * P], bf16, tag="tw1")
            if not split:     # scene 0's tw loads are sequenced by the caller
                nc.sync.dma_start(tw0[:], d_tw0[s])
                nc.sync.dma_start(tw1[:], d_tw1[s])
            obs_c = scene_pool.tile([P, 16], bf16, tag="obs_c")
            nc.sync.dma_start(obs_c[:], d_obs[s * P:(s + 1) * P, :])
            # obs_rep padded to K=128; rows P.. are zeroed (Dm rows are 0
            # there, but uninitialized SBUF could hold Inf/NaN patterns)
            obs_rep = scene_pool.tile([128, 128], bf16, tag="obs_rep")
            nc.vector.memset(obs_rep[:], 0.0)
            nc.vector.tensor_copy(
                obs_rep[:P].rearrange("p (r g) -> p r g", r=8),
                obs_c[:].unsqueeze(1).broadcast_to([P, 8, 16]))
            # h broadcast over i: hj_fm[:, ii*P + j] = h_fm[s, :, j];
            # replicated into both partition halves (W1h_sb holds W1h/2 twice)
            h_c = scene_pool.tile([128, P], bf16, tag="h_c")
            nc.sync.dma_start(h_c[:64], d_h[s])
            nc.sync.dma_start(h_c[64:], d_h[s])
            hj_fm = scene_pool.tile([128, NBLK], bf16, tag="hj_fm")
            nc.vector.tensor_copy(
                hj_fm[:].rearrange("p (r j) -> p r j", r=IB),
                h_c[:].unsqueeze(1).broadcast_to([128, IB, P]))
            pooled = scene_pool.tile([128, 8, P], f32, tag="pooled")
            state[s] = dict(tw0=tw0, tw1=tw1, obs_rep=obs_rep, hj_fm=hj_fm,
                            pooled=pooled)
            return tw0, tw1

        def prep(s, b):
            st = state[s]
            c0 = b * NBLK
            rel_ps = pp.tile([128, NBLK], f32, tag="pp")
            nc.tensor.matmul(rel_ps[:], st["obs_rep"][:],
                             Dm_sb[:, c0:c0 + NBLK], start=True, stop=True,
                             tile_position=(0, 0))
            rel2_0 = blk_pool.tile([128, NBLK], bf16, tag="rel2_0")
            nc.vector.tensor_tensor(rel2_0[:], st["tw0"][:, c0:c0 + NBLK],
                                    rel_ps[:], AluOpType.mult)
            rel2_1 = blk_pool.tile([128, NBLK], bf16, tag="rel2_1")
            nc.vector.tensor_tensor(rel2_1[:], st["tw1"][:, c0:c0 + NBLK],
                                    rel_ps[:], AluOpType.mult)
            return dict(rel2_0=rel2_0, rel2_1=rel2_1, s=s, b=b)

        def mlp1(job):
            st = state[job["s"]]
            r20 = job["rel2_0"][:]
            r21 = job["rel2_1"][:]
            x1 = blk_pool.tile([128, 4, NBLK], bf16, tag="x1")
            for m in range(4):
                p1t = p1.tile([128, NBLK], f32, tag="p1")
                nc.tensor.matmul(p1t[:], Wf_sb[:, 0, m * 128:(m + 1) * 128],
                                 r20, start=True, stop=False)
                nc.tensor.matmul(p1t[:], Wf_sb[:, 1, m * 128:(m + 1) * 128],
                                 r21, start=False, stop=False)
                nc.tensor.matmul(p1t[:], W1h_sb[:, m * 128:(m + 1) * 128],
                                 st["hj_fm"][:], start=False, stop=True)
                nc.scalar.activation(x1[:, m, :], p1t[:], AF.Relu,
                                     bias=b1_sb[:, m:m + 1])
            job["x1"] = x1

        def mlp2_mpair(job, mp, p2t=None):
            """MLP2 m-tiles 2*mp, 2*mp+1 of one block into a 2-bank PSUM
            pair tile, then one reduce covering both. Returns the pair tile
            for reuse by a sibling block."""
            s, b = job["s"], job["b"]
            x1 = job["x1"]
            if p2t is None:
                p2t = p2.tile([128, 2, 512], f32, tag="p2")
            for half in range(2):
                mm = 2 * mp + half
                for k in range(4):
                    nc.tensor.matmul(
                        p2t[:, half, :NBLK], W2_sb[:, k, mm * 128:(mm + 1) * 128],
                        x1[:, k, :], start=(k == 0), stop=(k == 3))
            nc.vector.tensor_reduce(
                state[s]["pooled"][:, 2 * mp:2 * mp + 2, b * IB:(b + 1) * IB],
                p2t[:, :, :NBLK].rearrange("p h (i j) -> p h i j", j=P),
                axis=AX.X, op=AluOpType.max)
            return p2t

        def finish_m(s, mm):
            # scene output for m-tile mm: bias+relu post-pool on the scalar
            # engine, then DMA the [128, P] slice (output is transposed)
            pooled = state[s]["pooled"]
            nc.scalar.activation(pooled[:, mm, :], pooled[:, mm, :],
                                 AF.Relu, bias=b2_sb[:, mm:mm + 1])
            nc.sync.dma_start(
                d_out[mm * 128:(mm + 1) * 128, s * P:(s + 1) * P],
                pooled[:, mm, :])

        def mlp2(job):
            s, b = job["s"], job["b"]
            last = b == NB - 1
            for mp in range(4):
                mlp2_mpair(job, mp)
                if last:
                    finish_m(s, 2 * mp)
                    finish_m(s, 2 * mp + 1)
            if last:
                state.pop(s)

        def mlp2_final_pair(job_a, job_b):
            """Last two blocks of the final scene, m-pair interleaved so the
            vector-engine reduce queue keeps pace with PE and the kernel
            tail is one m-pair deep instead of two blocks deep."""
            s = job_b["s"]
            for mp in range(4):
                mlp2_mpair(job_a, mp)
                mlp2_mpair(job_b, mp)
                finish_m(s, 2 * mp)
                finish_m(s, 2 * mp + 1)
            state.pop(s)

        # two-deep software pipeline on PE:
        #   ... prep(i)  mlp1(i-1)  mlp2(i-2) ...
        # so x1 is ready a full block before MLP2 consumes it; scene data is
        # prefetched two blocks before the scene starts; weights stream in
        # behind scene 0's data.
        # startup loads in critical-path order: Dm/Wf (first prep+mlp1),
        # scene-0 tw first halves (blocks 0-2), W1h, then W2 (first mlp2
        # needs all 4 chunks by ~6us), then the rest
        tw0_0, tw1_0 = scene_setup(0, split=True)
        HPP = P * P // 2
        nc.sync.dma_start(Dm_sb[:], d_Dm[:])
        nc.sync.dma_start(Wf_sb[:, 0], d_Wf[:, 0])
        nc.sync.dma_start(Wf_sb[:, 1], d_Wf[:, 1])
        nc.sync.dma_start(tw0_0[:, :HPP], d_tw0[0, :, :HPP])
        nc.sync.dma_start(tw1_0[:, :HPP], d_tw1[0, :, :HPP])
        nc.sync.dma_start(W1h_sb[:], d_W1h[:])
        nc.sync.dma_start(b1_sb[:], d_b1[:])
        for k in range(4):
            nc.sync.dma_start(W2_sb[:, k], d_W2[:, k])
        nc.sync.dma_start(b2_sb[:], d_b2[:])
        nc.sync.dma_start(tw0_0[:, HPP:], d_tw0[0, :, HPP:])
        nc.sync.dma_start(tw1_0[:, HPP:], d_tw1[0, :, HPP:])
        for s, b in blocks:
            if b == NB - 2 and s + 1 < n_scenes:
                scene_setup(s + 1)
            mlp_q.append(prep(s, b))
            if len(mlp_q) > 1:
                mlp1(mlp_q[-2])
            if len(mlp_q) > 2:
                mlp2(mlp_q.pop(0))
        mlp1(mlp_q[-1])
        mlp2_final_pair(mlp_q.pop(0), mlp_q.pop(0))

    nc.compile()
    return nc


def _host_inputs(h_states, traj, traj_weight, consts, n_scenes=SC):
    """Slice + lay out per-core input maps (matmul operands cast to bf16)."""
    import ml_dtypes
    bf = ml_dtypes.bfloat16
    h_states = np.asarray(h_states, np.float32)
    traj = np.asarray(traj, np.float32)
    traj_weight = np.asarray(traj_weight, np.float32)

    obs_full = np.ascontiguousarray(
        traj[:T].transpose(1, 0, 2).reshape(B, 2 * T))          # (B,16) g=t*2+c
    h_full = h_states.reshape(S, P, H)

    consts = dict(consts)
    for k in ("Wf_sb", "W1h_sb", "W2_sb", "Dm"):
        consts[k] = consts[k].astype(bf)

    in_maps = []
    for core in range(NCORES):
        s0 = core * n_scenes
        sl = slice(s0, s0 + n_scenes)
        twT = np.ascontiguousarray(
            traj_weight[sl].transpose(0, 2, 3, 1).reshape(n_scenes, 16, P * P))
        # pre-replicate each tw row 16x along partitions (feature-major)
        tw0r = np.ascontiguousarray(np.repeat(twT[:, 0:8], 16, axis=1)).astype(bf)
        tw1r = np.ascontiguousarray(np.repeat(twT[:, 8:16], 16, axis=1)).astype(bf)
        h_fm = np.ascontiguousarray(h_full[sl].transpose(0, 2, 1)).astype(bf)
        obs_rm = np.ascontiguousarray(
            obs_full[s0 * P:(s0 + n_scenes) * P]).astype(bf)
        m = dict(obs_rm=obs_rm, tw0r=tw0r, tw1r=tw1r, h_fm=h_fm)
        m.update(consts)
        in_maps.append(m)
    return in_maps


def kernel(h_states, seq_start_end, end_pos, traj, traj_weight,
           mlp_pre_pool_dim_0, W_se, b_se, W1, b1, W2, b2):
    import sys
    if '/opt/trn_rl_repo' not in sys.path:
        sys.path.insert(0, '/opt/trn_rl_repo')
    from concourse.bass_utils import run_bass_kernel_spmd

    consts = _host_constants(W_se, W1, W2, b1, b2)
    in_maps = _host_inputs(h_states, traj, traj_weight, consts)
    nc = build_program(SC)
    res = run_bass_kernel_spmd(nc, in_maps, list(range(NCORES)))
    out = np.concatenate(
        [res.results[i]["out"].T for i in range(NCORES)], axis=0)
    return np.ascontiguousarray(out).astype(np.float32)
